# revision 1
# baseline (speedup 1.0000x reference)
"""LoRA attention kernel for 8 Trainium2 NeuronCores.

Sharding: data-parallel over batch B=2 (cores 0-3 -> b=0, cores 4-7 -> b=1),
tensor-parallel over heads within each batch group (4 heads/core). The LoRA
low-rank paths and q/v base linears are folded host-side into one effective
qkv projection weight. Attention is computed with scores transposed
(ST[m, n]) so that the softmax denominator and key-padding mask both fold
into the P@V matmul via an augmented v column, and the P@V contraction runs
without transposing the (huge) probability matrix. The per-head attention
outputs (still transposed, [d, n]) are AllGathered across the 4-core group,
and each core computes a 256-row slice of the output projection.
"""

import sys
from contextlib import ExitStack

import numpy as np

for _p in ("/opt/trn_rl_repo", "/opt/trn_rl_repo/concourse"):
    if _p not in sys.path:
        sys.path.insert(0, _p)

import concourse.bass as bass
import concourse.mybir as mybir
import concourse.tile as tile
from concourse import bacc
from concourse import bass_utils
from concourse.masks import make_identity

F32 = mybir.dt.float32
F32R = mybir.dt.float32r
EXP = mybir.ActivationFunctionType.Exp

H, D, DIM, R = 16, 64, 1024, 10
B, N = 2, 2048
NCORES = 8
GROUPS = [[0, 1, 2, 3], [4, 5, 6, 7]]
HPC = H // 4          # heads per core
HD = HPC * D          # 256 qkv rows per core per projection
ATT = float(D) ** -0.5
LS = 1.0 / R

KT = DIM // 128       # 8 contraction tiles
NT = N // 128         # 16 token tiles
NCH = N // 512        # 4 moving chunks of 512
IT = (3 * HD) // 128  # 6 projection row tiles

# test harness hooks
TRACE = False
TRACE_DIR = None
LAST_RESULTS = None

_NC_CACHE = None


def _build_nc():
    nc = bacc.Bacc(None, target_bir_lowering=False, num_devices=NCORES)

    xT = nc.dram_tensor("xT", (DIM, N), F32R, kind="ExternalInput")
    wT = nc.dram_tensor("wT", (DIM, 3 * HD), F32R, kind="ExternalInput")
    pb = nc.dram_tensor("pb", (3 * HD,), F32, kind="ExternalInput")
    mk = nc.dram_tensor("mk", (N,), F32, kind="ExternalInput")
    woT = nc.dram_tensor("woT", (DIM, HD), F32R, kind="ExternalInput")
    bo = nc.dram_tensor("bo", (HD,), F32, kind="ExternalInput")
    outT = nc.dram_tensor("outT", (HD, N), F32, kind="ExternalOutput")

    agin = nc.dram_tensor("agin", (HD, N), F32R)
    agout = nc.dram_tensor("agout", (DIM, N), F32R)
    recd = nc.dram_tensor("recd", (HPC, N), F32)

    with ExitStack() as ctx:
        tc = ctx.enter_context(tile.TileContext(nc))
        const = ctx.enter_context(tc.tile_pool(name="const", bufs=1))

        ident_f32 = const.tile([128, 128], F32)
        make_identity(nc, ident_f32)
        ident = const.tile([128, 128], F32R)
        nc.vector.tensor_copy(ident, ident_f32)

        pb_sb = const.tile([128, IT], F32)
        nc.sync.dma_start(out=pb_sb, in_=pb[:].rearrange("(i p) -> p i", p=128))
        mk_sb = const.tile([128, NT], F32)
        nc.sync.dma_start(out=mk_sb, in_=mk[:].rearrange("(t p) -> p t", p=128))
        bo_sb = const.tile([128, HD // 128], F32)
        nc.sync.dma_start(out=bo_sb, in_=bo[:].rearrange("(c p) -> p c", p=128))
        woT_sb = const.tile([128, KT, HD], F32R)
        woT_r = woT[:, :].rearrange("(k p) c -> p k c", p=128)
        for k in range(KT):
            nc.sync.dma_start(out=woT_sb[:, k, :], in_=woT_r[:, k, :])

        qkvT = const.tile([128, IT, N], F32R)          # [q0..q255 | k | v] x n
        vsb = const.tile([128, NT, HPC, D + 1], F32R)  # v untransposed + mask col

        # ---- phase 1: fused qkv projection + v transpose/mask ----
        with tc.tile_pool(name="xw", bufs=1) as xw, \
             tc.tile_pool(name="pp_proj", bufs=4, space="PSUM") as ppp, \
             tc.tile_pool(name="pp_vt", bufs=2, space="PSUM") as ppvt:
            xT_sb = xw.tile([128, KT, N], F32R)
            wT_sb = xw.tile([128, KT, 3 * HD], F32R)
            wT_r = wT[:, :].rearrange("(k p) m -> p k m", p=128)
            xT_r = xT[:, :].rearrange("(k p) n -> p k n", p=128)
            for k in range(KT):
                nc.sync.dma_start(out=wT_sb[:, k, :], in_=wT_r[:, k, :])
            for k in range(KT):
                for half in range(2):
                    sl = slice(half * (N // 2), (half + 1) * (N // 2))
                    nc.sync.dma_start(out=xT_sb[:, k, sl], in_=xT_r[:, k, sl])

            for i in range(IT):
                pss = [ppp.tile([128, 512], F32, tag="ps", name=f"ps{i}_{_n}") for _n in range(NCH)]
                for k in range(KT):
                    lhs = wT_sb[:, k, i * 128:(i + 1) * 128]
                    for nch in range(NCH):
                        nc.tensor.matmul(
                            pss[nch],
                            lhsT=lhs,
                            rhs=xT_sb[:, k, nch * 512:(nch + 1) * 512],
                            start=(k == 0),
                            stop=(k == KT - 1),
                        )
                for nch in range(NCH):
                    nc.vector.tensor_scalar_add(
                        qkvT[:, i, nch * 512:(nch + 1) * 512],
                        pss[nch],
                        pb_sb[:, i:i + 1],
                    )

            # transpose vT -> v[m, d], zero masked rows, mask into aug column
            for t in range(NT):
                for j in range(2):
                    vt = ppvt.tile([128, 128], F32R, tag="vt", name=f"vt{t}_{j}")
                    nc.tensor.transpose(
                        vt, qkvT[:, 4 + j, t * 128:(t + 1) * 128], ident
                    )
                    for hh in range(2):
                        h = j * 2 + hh
                        nc.vector.tensor_scalar_mul(
                            vsb[:, t, h, 0:D],
                            vt[:, hh * 64:hh * 64 + 64],
                            mk_sb[:, t:t + 1],
                        )
                for h in range(HPC):
                    nc.vector.tensor_copy(vsb[:, t, h, D:D + 1], mk_sb[:, t:t + 1])

        # ---- phase 2: attention per head ----
        with tc.tile_pool(name="expool", bufs=6) as expool, \
             tc.tile_pool(name="attp", bufs=2) as attp, \
             tc.tile_pool(name="recbp", bufs=2) as recbp, \
             tc.tile_pool(name="recp", bufs=2) as recp, \
             tc.tile_pool(name="pp_o", bufs=1, space="PSUM") as ppo, \
             tc.tile_pool(name="pp_st", bufs=4, space="PSUM") as ppst:
            for h in range(HPC):
                ih, off = h // 2, (h % 2) * 64
                qTh = qkvT[off:off + 64, ih, :]
                kTh = qkvT[off:off + 64, 2 + ih, :]
                op = ppo.tile([128, N], F32, tag="op")
                for t in range(NT):
                    sts = []
                    lhs = kTh[:, t * 128:(t + 1) * 128]
                    for nch in range(NCH):
                        st = ppst.tile([128, 512], F32, tag="st", name=f"st{h}_{t}_{nch}")
                        nc.tensor.matmul(
                            st,
                            lhsT=lhs,
                            rhs=qTh[:, nch * 512:(nch + 1) * 512],
                            start=True,
                            stop=True,
                        )
                        sts.append(st)
                    exs = []
                    for nch in range(NCH):
                        ex = expool.tile([128, 512], F32R, tag="ex", name=f"ex{h}_{t}_{nch}")
                        nc.scalar.activation(ex, sts[nch], EXP)
                        exs.append(ex)
                    vlhs = vsb[:, t, h, :]
                    for nch in range(NCH):
                        nc.tensor.matmul(
                            op[0:D + 1, nch * 512:(nch + 1) * 512],
                            lhsT=vlhs,
                            rhs=exs[nch],
                            start=(t == 0),
                            stop=(t == NT - 1),
                        )
                # normalize rows 0..63 by reciprocal of denom row 64
                rec = recp.tile([1, N], F32, tag="rec")
                nc.vector.reciprocal(rec, op[D:D + 1, :])
                nc.sync.dma_start(out=recd[h:h + 1, :], in_=rec)
                recb = recbp.tile([64, N], F32, tag="recb")
                rsrc = recd[h:h + 1, :]
                nc.sync.dma_start(
                    out=recb,
                    in_=bass.AP(tensor=rsrc.tensor, offset=rsrc.offset,
                                ap=[[0, 64], [1, N]]),
                )
                att = attp.tile([64, N], F32R, tag="att")
                nc.vector.tensor_mul(att, op[0:D, :], recb)
                nc.sync.dma_start(out=agin[h * 64:(h + 1) * 64, :], in_=att)

        # ---- phase 3: AllGather heads within batch group ----
        nc.gpsimd.collective_compute(
            "AllGather",
            mybir.AluOpType.bypass,
            replica_groups=GROUPS,
            ins=[agin[:, :].opt()],
            outs=[agout[:, :].opt()],
        )

        # ---- phase 4: output projection slice ----
        with tc.tile_pool(name="agp", bufs=1) as agp, \
             tc.tile_pool(name="outp", bufs=2) as outp, \
             tc.tile_pool(name="pp_f", bufs=2, space="PSUM") as ppf:
            agT = agp.tile([128, KT, N], F32R)
            ag_r = agout[:, :].rearrange("(k p) n -> p k n", p=128)
            for k in range(KT):
                nc.sync.dma_start(out=agT[:, k, :], in_=ag_r[:, k, :])
            out_r = outT[:, :].rearrange("(c p) n -> p c n", p=128)
            for c in range(HD // 128):
                fp = ppf.tile([128, N], F32, tag="fp")
                for k in range(KT):
                    lhs = woT_sb[:, k, c * 128:(c + 1) * 128]
                    for nch in range(NCH):
                        nc.tensor.matmul(
                            fp[:, nch * 512:(nch + 1) * 512],
                            lhsT=lhs,
                            rhs=agT[:, k, nch * 512:(nch + 1) * 512],
                            start=(k == 0),
                            stop=(k == KT - 1),
                        )
                ot = outp.tile([128, N], F32, tag="ot")
                nc.vector.tensor_scalar_add(ot, fp, bo_sb[:, c:c + 1])
                nc.sync.dma_start(out=out_r[:, c, :], in_=ot)

    nc.finalize()
    return nc


def _prep_core_inputs(inputs, c):
    b, g = c // 4, c % 4
    rows = slice(g * HD, (g + 1) * HD)
    w_qkv = np.asarray(inputs["w_qkv"], np.float32)
    Wq = (w_qkv[0:H * D][rows]
          + np.asarray(inputs["wq_base"], np.float32)[rows]
          + LS * (np.asarray(inputs["wq_B"], np.float32)[rows]
                  @ np.asarray(inputs["wq_A"], np.float32))) * ATT
    Wk = w_qkv[H * D:2 * H * D][rows]
    Wv = (w_qkv[2 * H * D:3 * H * D][rows]
          + np.asarray(inputs["wv_base"], np.float32)[rows]
          + LS * (np.asarray(inputs["wv_B"], np.float32)[rows]
                  @ np.asarray(inputs["wv_A"], np.float32)))
    wTv = np.ascontiguousarray(np.concatenate([Wq, Wk, Wv], 0).T)
    pbv = np.concatenate([
        np.asarray(inputs["bq_base"], np.float32)[rows] * ATT,
        np.zeros(HD, np.float32),
        np.asarray(inputs["bv_base"], np.float32)[rows],
    ]).astype(np.float32)
    xTv = np.ascontiguousarray(np.asarray(inputs["x"], np.float32)[b].T)
    mkv = np.asarray(inputs["mask"]).astype(np.float32)[b]
    woTv = np.ascontiguousarray(
        np.asarray(inputs["w_out"], np.float32)[rows, :].T)
    bov = np.asarray(inputs["b_out"], np.float32)[rows]
    return {"xT": xTv, "wT": wTv, "pb": pbv, "mk": mkv, "woT": woTv, "bo": bov}


def kernel(**inputs):
    global _NC_CACHE, LAST_RESULTS
    if _NC_CACHE is None:
        _NC_CACHE = _build_nc()
    nc = _NC_CACHE
    in_maps = [_prep_core_inputs(inputs, c) for c in range(NCORES)]
    res = bass_utils.run_bass_kernel_spmd(
        nc, in_maps, core_ids=list(range(NCORES)),
        trace=TRACE, tmpdir=TRACE_DIR,
    )
    LAST_RESULTS = res
    out = np.empty((B, N, DIM), np.float32)
    for c in range(NCORES):
        b, g = c // 4, c % 4
        out[b, :, g * HD:(g + 1) * HD] = res.results[c]["outT"].T
    return out



# revision 14
# speedup vs baseline: 2.7696x; 2.7696x over previous
"""LoRA attention kernel for 8 Trainium2 NeuronCores.

Sharding: data-parallel over batch B=2 (cores 0-3 -> b=0, cores 4-7 -> b=1),
tensor-parallel over heads within each batch group (4 heads/core). LoRA paths
and q/v base linears are folded host-side into effective projection weights.

Key optimizations over the fp32r baseline:
- All matmul operands are bf16 (PSUM accumulation stays fp32); tolerance is
  2e-2 so bf16 rounding (~1e-3) is safe and weight loads/streams/DMAs halve.
- Key/value tokens are compacted host-side using the padding mask (roughly
  half of the 2048 keys are masked out), halving QK/exp/PV work. Padded tail
  keys carry mask 0 and contribute exactly zero, so results are unchanged.
- Scores are computed transposed (ST[m, n]) so the softmax denominator and
  key mask fold into the P@V matmul via an augmented v column. The attention
  inner loop runs on [128, 1024] double-buffered PSUM tiles (2 banks each,
  2x st + 2x op = 8 banks) so PE, ACT (exp), and DVE all overlap.
- 1/denominator uses the fast approximate DVE reciprocal (~5x faster than
  the exact one, which previously stalled the PE ~13us per head).
- The per-head attention outputs are AllGathered in bf16 one head at a time,
  overlapping the collectives with attention compute of later heads; only
  the last head's AllGather is exposed.
- The output projection contracts the AllGathered heads (in gather order,
  with w_out rows permuted host-side to match) right after the last gather.
"""

import sys
from contextlib import ExitStack

import numpy as np

for _p in ("/opt/trn_rl_repo", "/opt/trn_rl_repo/concourse"):
    if _p not in sys.path:
        sys.path.insert(0, _p)

import concourse.bass as bass
import concourse.mybir as mybir
import concourse.tile as tile
from concourse import bacc
from concourse import bass_utils
from concourse.masks import make_identity

import ml_dtypes

F32 = mybir.dt.float32
BF16 = mybir.dt.bfloat16
EXP = mybir.ActivationFunctionType.Exp
BFNP = ml_dtypes.bfloat16

H, D, DIM, R = 16, 64, 1024, 10
B, N = 2, 2048
NCORES = 8
GROUPS = [[0, 1, 2, 3], [4, 5, 6, 7]]
HPC = H // 4          # heads per core
HD = HPC * D          # 256 projection rows per core
ATT = float(D) ** -0.5
LS = 1.0 / R

KT = DIM // 128       # 8 contraction tiles
NCH = N // 512        # 4 query chunks of 512
# AllGather head order: agout[h] rows are rank-major, so the out-projection
# contraction sees global heads in this order (w_out rows permuted to match).
AG_HEAD_ORDER = [r * HPC + h for h in range(HPC) for r in range(4)]

# test harness hooks
TRACE = False
TRACE_DIR = None
LAST_RESULTS = None

_NC_CACHE = {}


def _chunks(total, step):
    return [(c0, min(c0 + step, total)) for c0 in range(0, total, step)]


def _build_nc(M, debug=False):
    MT = M // 128
    MCH = _chunks(M, 512)
    dbg = "ExternalOutput" if debug else "Internal"

    nc = bacc.Bacc(None, target_bir_lowering=False, num_devices=NCORES)

    xT = nc.dram_tensor("xT", (DIM, N), BF16, kind="ExternalInput")
    xkT = nc.dram_tensor("xkT", (DIM, M), BF16, kind="ExternalInput")
    wqT = nc.dram_tensor("wqT", (DIM, HD), BF16, kind="ExternalInput")
    wkvT = nc.dram_tensor("wkvT", (DIM, 2 * HD), BF16, kind="ExternalInput")
    pbq = nc.dram_tensor("pbq", (HD,), F32, kind="ExternalInput")
    pbv = nc.dram_tensor("pbv", (HD,), F32, kind="ExternalInput")
    mk = nc.dram_tensor("mk", (M,), F32, kind="ExternalInput")
    woT = nc.dram_tensor("woT", (DIM, HD), BF16, kind="ExternalInput")
    bo = nc.dram_tensor("bo", (HD,), F32, kind="ExternalInput")
    outT = nc.dram_tensor("outT", (HD, N), F32, kind="ExternalOutput")

    agin = nc.dram_tensor("agin", (HPC * D, N), BF16, kind="ExternalOutput" if debug == "agin" else "Internal")
    agout = nc.dram_tensor("agout", (HPC, 4 * D, N), BF16)
    recd = nc.dram_tensor("recd", (HPC, N), F32, kind="ExternalOutput" if debug == "recd" else "Internal")

    with ExitStack() as ctx:
        tc = ctx.enter_context(tile.TileContext(nc))
        const = ctx.enter_context(tc.tile_pool(name="const", bufs=1))

        ident_f32 = const.tile([128, 128], F32)
        make_identity(nc, ident_f32)
        ident = const.tile([128, 128], BF16)
        nc.vector.tensor_copy(ident, ident_f32)

        pbq_sb = const.tile([128, 2], F32)
        nc.sync.dma_start(out=pbq_sb, in_=pbq[:].rearrange("(i p) -> p i", p=128))
        pbv_sb = const.tile([128, 2], F32)
        nc.sync.dma_start(out=pbv_sb, in_=pbv[:].rearrange("(i p) -> p i", p=128))
        mk_sb = const.tile([128, MT], F32)
        nc.sync.dma_start(out=mk_sb, in_=mk[:].rearrange("(t p) -> p t", p=128))
        bo_sb = const.tile([128, 2], F32)
        nc.sync.dma_start(out=bo_sb, in_=bo[:].rearrange("(c p) -> p c", p=128))
        woT_sb = const.tile([128, KT, HD], BF16)
        woT_r = woT[:, :].rearrange("(k p) c -> p k c", p=128)
        for k in range(KT):
            nc.sync.dma_start(out=woT_sb[:, k, :], in_=woT_r[:, k, :])

        # q in zero-padded per-head layout: head h occupies partitions
        # (h%2)*64..+64 of qz[:, h, :], the other half stays zero. QK can then
        # contract over all 128 partitions (full-rate; K=64 matmuls stream at
        # half rate) against the head-pair K tile — the zero half adds 0.
        qz = const.tile([128, HPC, N], BF16)
        nc.vector.memset(qz, 0)
        kvT = const.tile([128, 4, M], BF16)      # [k0 k1 v0 v1] row tiles
        vsb = const.tile([128, MT, HPC, D + 1], BF16)  # v.T + mask column
        agT = const.tile([128, KT, N], BF16)     # gathered heads for out proj

        # ---- phase 1: projections (K first so attention can start early) ----
        with tc.tile_pool(name="xw", bufs=1) as xw, \
             tc.tile_pool(name="pp_proj", bufs=4, space="PSUM") as ppp, \
             tc.tile_pool(name="pp_vt", bufs=2, space="PSUM") as ppvt:
            wq_sb = xw.tile([128, KT, HD], BF16)
            wkv_sb = xw.tile([128, KT, 2 * HD], BF16)
            xk_sb = xw.tile([128, KT, M], BF16)
            xT_sb = xw.tile([128, KT, N], BF16)
            wq_r = wqT[:, :].rearrange("(k p) m -> p k m", p=128)
            wkv_r = wkvT[:, :].rearrange("(k p) m -> p k m", p=128)
            xk_r = xkT[:, :].rearrange("(k p) n -> p k n", p=128)
            xT_r = xT[:, :].rearrange("(k p) n -> p k n", p=128)
            for k in range(KT):
                nc.sync.dma_start(out=wkv_sb[:, k, :], in_=wkv_r[:, k, :])
                nc.sync.dma_start(out=wq_sb[:, k, :], in_=wq_r[:, k, :])
                nc.sync.dma_start(out=xk_sb[:, k, :], in_=xk_r[:, k, :])
            for k in range(KT):
                for half in range(2):
                    sl = slice(half * (N // 2), (half + 1) * (N // 2))
                    nc.sync.dma_start(out=xT_sb[:, k, sl], in_=xT_r[:, k, sl])

            # K projection (no bias)
            for i in range(2):
                for ci, (c0, c1) in enumerate(MCH):
                    ps = ppp.tile([128, 512], F32, tag="ps", name=f"psk{i}_{ci}")
                    for k in range(KT):
                        nc.tensor.matmul(
                            ps[:, 0:c1 - c0],
                            lhsT=wkv_sb[:, k, i * 128:(i + 1) * 128],
                            rhs=xk_sb[:, k, c0:c1],
                            start=(k == 0), stop=(k == KT - 1),
                        )
                    nc.vector.tensor_copy(kvT[:, i, c0:c1], ps[:, 0:c1 - c0])
            # Q projection (+bias, ATT-scaled host-side); the row tile covers
            # heads 2i and 2i+1 — the bias-add scatters each head's 64 rows
            # into its zero-padded qz slot.
            for i in range(2):
                for nch in range(NCH):
                    sl = slice(nch * 512, (nch + 1) * 512)
                    ps = ppp.tile([128, 512], F32, tag="ps", name=f"psq{i}_{nch}")
                    for k in range(KT):
                        nc.tensor.matmul(
                            ps,
                            lhsT=wq_sb[:, k, i * 128:(i + 1) * 128],
                            rhs=xT_sb[:, k, sl],
                            start=(k == 0), stop=(k == KT - 1),
                        )
                    nc.vector.tensor_scalar_add(
                        qz[0:64, 2 * i, sl], ps[0:64, :], pbq_sb[0:64, i:i + 1]
                    )
                    nc.vector.tensor_scalar_add(
                        qz[64:128, 2 * i + 1, sl], ps[64:128, :],
                        pbq_sb[64:128, i:i + 1]
                    )
            # V projection (+bias)
            for i in range(2):
                for ci, (c0, c1) in enumerate(MCH):
                    ps = ppp.tile([128, 512], F32, tag="ps", name=f"psv{i}_{ci}")
                    for k in range(KT):
                        nc.tensor.matmul(
                            ps[:, 0:c1 - c0],
                            lhsT=wkv_sb[:, k, HD + i * 128:HD + (i + 1) * 128],
                            rhs=xk_sb[:, k, c0:c1],
                            start=(k == 0), stop=(k == KT - 1),
                        )
                    nc.vector.tensor_scalar_add(
                        kvT[:, 2 + i, c0:c1], ps[:, 0:c1 - c0], pbv_sb[:, i:i + 1]
                    )
            # transpose v -> v[m, d], zero masked/pad rows, mask aug column
            for t in range(MT):
                for j in range(2):
                    vt = ppvt.tile([128, 128], BF16, tag="vt", name=f"vt{t}_{j}")
                    nc.tensor.transpose(
                        vt, kvT[:, 2 + j, t * 128:(t + 1) * 128], ident
                    )
                    for hh in range(2):
                        h = j * 2 + hh
                        nc.vector.tensor_scalar_mul(
                            vsb[:, t, h, 0:D],
                            vt[:, hh * D:(hh + 1) * D],
                            mk_sb[:, t:t + 1],
                        )
                for h in range(HPC):
                    nc.vector.tensor_copy(vsb[:, t, h, D:D + 1], mk_sb[:, t:t + 1])

        # ---- phase 2: attention per head, AllGather pipelined per head ----
        with tc.tile_pool(name="expool", bufs=4) as expool, \
             tc.tile_pool(name="attp", bufs=2) as attp, \
             tc.tile_pool(name="recbp", bufs=2) as recbp, \
             tc.tile_pool(name="recp", bufs=2) as recp, \
             tc.tile_pool(name="pp_o", bufs=2, space="PSUM") as ppo, \
             tc.tile_pool(name="pp_st", bufs=2, space="PSUM") as ppst:
            for h in range(HPC):
                ih = h // 2
                qTh = qz[:, h, :]
                kTh = kvT[:, ih, :]
                for half in range(2):
                    nsl = slice(half * 1024, (half + 1) * 1024)
                    op = ppo.tile([D + 1, 1024], F32, tag="op",
                                  name=f"op{h}_{half}")

                    def pv(t, ext):
                        for c in range(2):
                            nc.tensor.matmul(
                                op[:, c * 512:(c + 1) * 512],
                                lhsT=vsb[:, t, h, :],
                                rhs=ext[:, c * 512:(c + 1) * 512],
                                start=(t == 0), stop=(t == MT - 1),
                            )

                    # PV lags QK by one tile so the in-order PE always has a
                    # ready QK to run while ACT computes the current exp.
                    exs = []
                    for t in range(MT):
                        st = ppst.tile([128, 1024], F32, tag="st",
                                       name=f"st{h}_{half}_{t}")
                        for c in range(2):
                            nc.tensor.matmul(
                                st[:, c * 512:(c + 1) * 512],
                                lhsT=kTh[:, t * 128:(t + 1) * 128],
                                rhs=qTh[:, half * 1024 + c * 512:
                                        half * 1024 + (c + 1) * 512],
                                start=True, stop=True,
                            )
                        ex = expool.tile([128, 1024], BF16, tag="ex",
                                         name=f"ex{h}_{half}_{t}")
                        nc.scalar.activation(ex, st, EXP)
                        exs.append(ex)
                        if t > 0:
                            pv(t - 1, exs[t - 1])
                    pv(MT - 1, exs[MT - 1])
                    den = recp.tile([1, 1024], F32, tag="den",
                                    name=f"den{h}_{half}")
                    nc.vector.tensor_copy(den, op[D:D + 1, :])
                    rec = recp.tile([1, 1024], F32, tag="rec",
                                    name=f"rec{h}_{half}")
                    nc.vector.reciprocal_approx_fast(rec, den)
                    nc.sync.dma_start(out=recd[h:h + 1, nsl], in_=rec)
                    recb = recbp.tile([D, 1024], F32, tag="recb",
                                      name=f"recb{h}_{half}")
                    rsrc = recd[h:h + 1, nsl]
                    nc.sync.dma_start(
                        out=recb,
                        in_=bass.AP(tensor=rsrc.tensor, offset=rsrc.offset,
                                    ap=[[0, D], [1, 1024]]),
                    )
                    att = attp.tile([D, 1024], BF16, tag="att",
                                    name=f"att{h}_{half}")
                    nc.vector.tensor_mul(att, op[0:D, :], recb)
                    nc.sync.dma_start(out=agin[h * D:(h + 1) * D, nsl], in_=att)
                nc.gpsimd.collective_compute(
                    "AllGather", mybir.AluOpType.bypass,
                    replica_groups=GROUPS,
                    ins=[agin[h * D:(h + 1) * D, :].opt()],
                    outs=[agout[h, :, :].opt()],
                )
                ag_r = agout[h, :, :].rearrange("(kk p) n -> p kk n", p=128)
                for kk in range(2):
                    nc.sync.dma_start(out=agT[:, 2 * h + kk, :], in_=ag_r[:, kk, :])

        # ---- phase 3: output projection slice ----
        with tc.tile_pool(name="outp", bufs=2) as outp, \
             tc.tile_pool(name="pp_f", bufs=2, space="PSUM") as ppf:
            out_r = outT[:, :].rearrange("(c p) n -> p c n", p=128)
            for c in range(2):
                fp = ppf.tile([128, N], F32, tag="fp", name=f"fp{c}")
                for k in range(KT):
                    lhs = woT_sb[:, k, c * 128:(c + 1) * 128]
                    for nch in range(NCH):
                        nc.tensor.matmul(
                            fp[:, nch * 512:(nch + 1) * 512],
                            lhsT=lhs,
                            rhs=agT[:, k, nch * 512:(nch + 1) * 512],
                            start=(k == 0), stop=(k == KT - 1),
                        )
                ot = outp.tile([128, N], F32, tag="ot", name=f"ot{c}")
                nc.vector.tensor_scalar_add(ot, fp, bo_sb[:, c:c + 1])
                nc.sync.dma_start(out=out_r[:, c, :], in_=ot)

    nc.finalize()
    return nc


def _pad_len(n):
    return max(128, ((n + 127) // 128) * 128)


def _prep_core_inputs(inputs, c, M, idxs):
    b, g = c // 4, c % 4
    rows = slice(g * HD, (g + 1) * HD)
    w_qkv = np.asarray(inputs["w_qkv"], np.float32)
    Wq = (w_qkv[0:H * D][rows]
          + np.asarray(inputs["wq_base"], np.float32)[rows]
          + LS * (np.asarray(inputs["wq_B"], np.float32)[rows]
                  @ np.asarray(inputs["wq_A"], np.float32))) * ATT
    Wk = w_qkv[H * D:2 * H * D][rows]
    Wv = (w_qkv[2 * H * D:3 * H * D][rows]
          + np.asarray(inputs["wv_base"], np.float32)[rows]
          + LS * (np.asarray(inputs["wv_B"], np.float32)[rows]
                  @ np.asarray(inputs["wv_A"], np.float32)))
    wqTv = np.ascontiguousarray(Wq.T).astype(BFNP)
    wkvTv = np.ascontiguousarray(np.concatenate([Wk, Wv], 0).T).astype(BFNP)
    pbqv = (np.asarray(inputs["bq_base"], np.float32)[rows] * ATT).astype(np.float32)
    pbvv = np.asarray(inputs["bv_base"], np.float32)[rows]

    xb = np.asarray(inputs["x"], np.float32)[b]          # [N, DIM]
    xTv = np.ascontiguousarray(xb.T).astype(BFNP)
    idx = idxs[b]
    xk = np.zeros((DIM, M), np.float32)
    xk[:, :len(idx)] = xb[idx].T
    xkTv = xk.astype(BFNP)
    mkv = np.zeros(M, np.float32)
    mkv[:len(idx)] = 1.0

    w_out_slice = np.asarray(inputs["w_out"], np.float32)[rows, :]   # [256, 1024]
    cols = np.concatenate([np.arange(gh * D, (gh + 1) * D)
                           for gh in AG_HEAD_ORDER])
    woTv = np.ascontiguousarray(w_out_slice[:, cols].T).astype(BFNP)
    bov = np.asarray(inputs["b_out"], np.float32)[rows]
    return {"xT": xTv, "xkT": xkTv, "wqT": wqTv, "wkvT": wkvTv,
            "pbq": pbqv, "pbv": pbvv, "mk": mkv, "woT": woTv, "bo": bov}


def kernel(**inputs):
    global LAST_RESULTS
    mask = np.asarray(inputs["mask"]).astype(bool)
    idxs = [np.nonzero(mask[b])[0] for b in range(B)]
    M = _pad_len(max(len(ix) for ix in idxs))
    if M not in _NC_CACHE:
        _NC_CACHE[M] = _build_nc(M)
    nc = _NC_CACHE[M]
    in_maps = [_prep_core_inputs(inputs, c, M, idxs) for c in range(NCORES)]
    res = bass_utils.run_bass_kernel_spmd(
        nc, in_maps, core_ids=list(range(NCORES)),
        trace=TRACE, tmpdir=TRACE_DIR,
    )
    LAST_RESULTS = res
    out = np.empty((B, N, DIM), np.float32)
    for c in range(NCORES):
        b, g = c // 4, c % 4
        out[b, :, g * HD:(g + 1) * HD] = res.results[c]["outT"].T
    return out
